# revision 9
# baseline (speedup 1.0000x reference)
"""Single-head attention (B=16, N=2048, d_in=256, d_qk=d_v=64) on 8 TRN2
NeuronCores, data-parallel over batch (2 batches per core, no collectives).

Math per batch b:
  q = x@wq + bq ; k = x@wk + bk ; v = x@wv + bv
  out = softmax(q k^T / 8) v

Device layout choices:
  - host feeds x^T (d on partitions) so every matmul has its contraction
    dim on partitions with zero on-device transposes
  - scores are computed TRANSPOSED: ST[m(keys) partitions, n(queries) free]
    so that P = exp(ST) is directly the rhs of the attention@V matmul
    (lhsT = V[m, dv]); exp needs no max-subtraction (scores sigma~0.6)
  - score matmuls have K=64 so PAIRS of key-chunks are row-packed into the
    two halves of the 128x128 PE array (tile_position via base partition);
    Q^T/K^T are duplicated into both partition halves to support this
  - per query-half: phase A streams score-pairs through exp into stored
    P tiles (ScalarE-paced); phase B runs the attention@V matmuls
    back-to-back (dense PE burst, overlaps next half's phase A)
  - wv is augmented with a 65th column of ones (via the bias row) so the
    softmax denominator appears as row 64 of the output accumulator
  - denominator reciprocal is broadcast across partitions with a ones
    matmul; final output is written as out^T [dv, n] and transposed on host
"""

import os
from contextlib import ExitStack

import numpy as np

N_CORES = 8
B, N, D_IN, D_QK, D_V = 16, 2048, 256, 64, 64
BPC = B // N_CORES  # batches per core
EV1 = D_V + 1  # v augmented with ones column (softmax denominator)
NH = 1024  # query-dim width of an ot psum tile (2 banks)

_CACHE = {}

# exec time of the most recent profiled run (test harness convenience)
LAST_EXEC_TIME_NS = None


def _build_nc():
    import concourse.tile as tile
    from concourse import bacc, mybir

    f32 = mybir.dt.float32
    bf16 = mybir.dt.bfloat16
    Exp = mybir.ActivationFunctionType.Exp

    nc = bacc.Bacc(
        "TRN2", target_bir_lowering=False, debug=False,
        enable_asserts=True, num_devices=N_CORES,
    )

    xt = nc.dram_tensor("xt", [BPC, D_IN, N], bf16, kind="ExternalInput").ap()
    wq = nc.dram_tensor("wq", [D_IN, D_QK], bf16, kind="ExternalInput").ap()
    wk = nc.dram_tensor("wk", [D_IN, D_QK], bf16, kind="ExternalInput").ap()
    wv = nc.dram_tensor("wv", [D_IN, EV1], bf16, kind="ExternalInput").ap()
    bq = nc.dram_tensor("bq", [D_QK, 1], f32, kind="ExternalInput").ap()
    bk = nc.dram_tensor("bk", [D_QK, 1], f32, kind="ExternalInput").ap()
    bv = nc.dram_tensor("bv", [128, EV1], f32, kind="ExternalInput").ap()
    ones = nc.dram_tensor("ones", [1, D_V], bf16, kind="ExternalInput").ap()
    out = nc.dram_tensor("out", [BPC, D_V, N], f32, kind="ExternalOutput").ap()

    with tile.TileContext(nc) as tc, ExitStack() as ctx:
        consts = ctx.enter_context(tc.tile_pool(name="consts", bufs=1))
        xt_pool = ctx.enter_context(tc.tile_pool(name="xt", bufs=2))
        qk_pool = ctx.enter_context(tc.tile_pool(name="qk", bufs=2))
        v_pool = ctx.enter_context(tc.tile_pool(name="v", bufs=2))
        p_pool = ctx.enter_context(tc.tile_pool(name="p", bufs=34))
        rb_pool = ctx.enter_context(tc.tile_pool(name="rb", bufs=2))
        small = ctx.enter_context(tc.tile_pool(name="small", bufs=3))
        outp = ctx.enter_context(tc.tile_pool(name="outp", bufs=3))
        # one shared PSUM pool: 4 slots x 2 banks = all 8 banks
        psum = ctx.enter_context(tc.tile_pool(name="psum", bufs=4, space="PSUM"))

        with nc.allow_low_precision(reason="bf16 attention intermediates"):
            # ---- constants (loaded once) ----
            wq_sb = consts.tile([128, 2 * D_QK], bf16, tag="wq")
            wk_sb = consts.tile([128, 2 * D_QK], bf16, tag="wk")
            wv_sb = consts.tile([128, 2 * EV1], bf16, tag="wv")
            for kk in range(2):
                nc.sync.dma_start(
                    wq_sb[:, kk * D_QK:(kk + 1) * D_QK], wq[kk * 128:(kk + 1) * 128, :])
                nc.sync.dma_start(
                    wk_sb[:, kk * D_QK:(kk + 1) * D_QK], wk[kk * 128:(kk + 1) * 128, :])
                nc.sync.dma_start(
                    wv_sb[:, kk * EV1:(kk + 1) * EV1], wv[kk * 128:(kk + 1) * 128, :])
            bq_sb = consts.tile([D_QK, 1], f32, tag="bq")
            bk_sb = consts.tile([D_QK, 1], f32, tag="bk")
            bv_sb = consts.tile([128, EV1], f32, tag="bv")
            ones_sb = consts.tile([1, D_V], bf16, tag="ones")
            nc.sync.dma_start(bq_sb[:], bq[:, :])
            nc.sync.dma_start(bk_sb[:], bk[:, :])
            nc.sync.dma_start(bv_sb[:], bv[:, :])
            nc.sync.dma_start(ones_sb[:], ones[:, :])
            # warm the exp table during the initial DMAs
            warm = small.tile([1, 1], f32, tag="warm")
            nc.scalar.activation(warm[:], bq_sb[0:1, 0:1], Exp)

            for b in range(BPC):
                # ---- load x^T (two 128-row d-tiles, split for early start) ----
                xt_sb = xt_pool.tile([128, 2 * N], bf16, tag="xt")
                for kk in range(2):
                    for hh in range(2):
                        nc.sync.dma_start(
                            xt_sb[:, kk * N + hh * NH: kk * N + (hh + 1) * NH],
                            xt[b, kk * 128:(kk + 1) * 128, hh * NH:(hh + 1) * NH])

                # ---- QT/KT = w^T x^T + bias, duplicated into both halves ----
                # ktile-outer so each weight load serves two 512-col matmuls
                qtd = qk_pool.tile([128, N], bf16, tag="qt")
                ktd = qk_pool.tile([128, N], bf16, tag="kt")
                for w_sb, b_sb, dst in ((wq_sb, bq_sb, qtd), (wk_sb, bk_sb, ktd)):
                    for jh in range(2):  # 1024-wide column group
                        ps = psum.tile([D_QK, NH], f32, tag="big")
                        for kk in range(2):
                            for j in range(2):
                                nc.tensor.matmul(
                                    ps[:, j * 512:(j + 1) * 512],
                                    w_sb[:, kk * D_QK:(kk + 1) * D_QK],
                                    xt_sb[:, kk * N + jh * NH + j * 512:
                                          kk * N + jh * NH + (j + 1) * 512],
                                    start=(kk == 0), stop=(kk == 1))
                        for j in range(2):
                            jj = jh * NH + j * 512
                            nc.vector.tensor_scalar_add(
                                dst[0:D_QK, jj:jj + 512],
                                ps[:, j * 512:(j + 1) * 512], b_sb[:])
                            # duplicate into partitions 64..127 (row-packing)
                            nc.sync.dma_start(
                                dst[D_QK:128, jj:jj + 512], dst[0:D_QK, jj:jj + 512])

                # ---- V_aug = x wv_aug + bv_aug  [m 128, 65] bf16, 16 tiles ----
                v_sb = v_pool.tile([128, 16 * EV1], bf16, tag="v")
                for m in range(16):
                    ps = psum.tile([128, EV1], f32, tag="big")
                    for kk in range(2):
                        nc.tensor.matmul(
                            ps[:],
                            xt_sb[:, kk * N + m * 128: kk * N + (m + 1) * 128],
                            wv_sb[:, kk * EV1:(kk + 1) * EV1],
                            start=(kk == 0), stop=(kk == 1))
                    nc.vector.tensor_add(
                        v_sb[:, m * EV1:(m + 1) * EV1], ps[:], bv_sb[:])

                # ---- attention, per query-half of 1024 ----
                for h in range(N // NH):
                    # phase A: scores + exp, P stored (ScalarE-paced)
                    p_tiles = []
                    for mp in range(8):  # key-chunk pair (2*mp, 2*mp+1)
                        m0, m1 = 2 * mp, 2 * mp + 1
                        pjs = []
                        for js in range(2):  # 512-wide query slice in half
                            q0 = h * NH + js * 512
                            st = psum.tile([128, NH], f32, tag="big")
                            # row-packed pair: array rows 0-63 and 64-127
                            nc.tensor.matmul(
                                st[:, 0:512],
                                ktd[0:D_QK, m0 * 128:(m0 + 1) * 128],
                                qtd[0:D_QK, q0:q0 + 512],
                                start=True, stop=True)
                            nc.tensor.matmul(
                                st[:, 512:1024],
                                ktd[D_QK:128, m1 * 128:(m1 + 1) * 128],
                                qtd[D_QK:128, q0:q0 + 512],
                                start=True, stop=True)
                            p = p_pool.tile([128, NH], bf16, tag="p")
                            nc.scalar.activation(p[:], st[:], Exp)
                            pjs.append(p)
                        p_tiles.append(pjs)

                    # phase B: attention @ V_aug, dense PE burst;
                    # weight (v[m]) shared by consecutive matmuls
                    ot = psum.tile([EV1, NH], f32, tag="big")
                    for mp in range(8):
                        m0, m1 = 2 * mp, 2 * mp + 1
                        p0, p1 = p_tiles[mp]
                        nc.tensor.matmul(
                            ot[:, 0:512], v_sb[:, m0 * EV1:(m0 + 1) * EV1],
                            p0[:, 0:512],
                            start=(mp == 0), stop=False, skip_group_check=True)
                        nc.tensor.matmul(
                            ot[:, 512:1024], v_sb[:, m0 * EV1:(m0 + 1) * EV1],
                            p1[:, 0:512],
                            start=(mp == 0), stop=False, skip_group_check=True)
                        nc.tensor.matmul(
                            ot[:, 0:512], v_sb[:, m1 * EV1:(m1 + 1) * EV1],
                            p0[:, 512:1024],
                            start=False, stop=(mp == 7), skip_group_check=True)
                        nc.tensor.matmul(
                            ot[:, 512:1024], v_sb[:, m1 * EV1:(m1 + 1) * EV1],
                            p1[:, 512:1024],
                            start=False, stop=(mp == 7), skip_group_check=True)

                    # ---- epilogue: divide by denominator (row 64), store ----
                    den_sb = small.tile([1, NH], f32, tag="den")
                    nc.vector.tensor_copy(den_sb[:], ot[D_V:EV1, :])
                    rcp = small.tile([1, NH], f32, tag="rcp")
                    nc.vector.reciprocal_approx_fast(rcp[:], den_sb[:])
                    rcp16 = small.tile([1, NH], bf16, tag="rcp16")
                    nc.vector.tensor_copy(rcp16[:], rcp[:])
                    rb_sb = rb_pool.tile([D_V, NH], bf16, tag="rb")
                    for j in range(NH // 512):
                        rb_ps = psum.tile([D_V, 512], f32, tag="big")
                        nc.tensor.matmul(
                            rb_ps[:], ones_sb[:],
                            rcp16[:, j * 512:(j + 1) * 512],
                            start=True, stop=True)
                        nc.vector.tensor_copy(rb_sb[:, j * 512:(j + 1) * 512], rb_ps[:])
                    o_sb = outp.tile([D_V, NH], f32, tag="o")
                    nc.vector.tensor_mul(o_sb[:], ot[0:D_V, :], rb_sb[:])
                    nc.sync.dma_start(out[b, :, h * NH:(h + 1) * NH], o_sb[:])

    nc.compile()
    return nc


def _get_nc():
    if "nc" not in _CACHE:
        _CACHE["nc"] = _build_nc()
    return _CACHE["nc"]


def kernel(x, wq, bq, wk, bk, wv, bv):
    global LAST_EXEC_TIME_NS
    import ml_dtypes
    from concourse.bass_utils import run_bass_kernel_spmd

    bf16 = ml_dtypes.bfloat16
    x = np.asarray(x, dtype=np.float32)
    wq = np.asarray(wq, dtype=np.float32)
    wk = np.asarray(wk, dtype=np.float32)
    wv = np.asarray(wv, dtype=np.float32)
    bq = np.asarray(bq, dtype=np.float32)
    bk = np.asarray(bk, dtype=np.float32)
    bv = np.asarray(bv, dtype=np.float32)

    # host-side layout/precision prep
    xt = np.ascontiguousarray(x.transpose(0, 2, 1)).astype(bf16)  # [B, D, N]
    wq8 = (wq / 8.0).astype(bf16)  # fold 1/sqrt(d_qk) into q projection
    bq8 = (bq / 8.0).astype(np.float32).reshape(D_QK, 1)
    wkb = wk.astype(bf16)
    bkb = bk.astype(np.float32).reshape(D_QK, 1)
    wv_aug = np.zeros((D_IN, EV1), np.float32)
    wv_aug[:, :D_V] = wv
    wv_augb = wv_aug.astype(bf16)
    bv_aug = np.zeros((128, EV1), np.float32)
    bv_aug[:, :D_V] = bv  # broadcast bias to all partitions
    bv_aug[:, D_V] = 1.0  # ones column -> softmax denominator
    ones = np.ones((1, D_V), bf16)

    in_maps = []
    for c in range(N_CORES):
        in_maps.append({
            "xt": np.ascontiguousarray(xt[BPC * c: BPC * (c + 1)]),
            "wq": wq8, "wk": wkb, "wv": wv_augb,
            "bq": bq8, "bk": bkb, "bv": bv_aug, "ones": ones,
        })

    nc = _get_nc()
    trace = bool(int(os.environ.get("ATTN_PROFILE", "0")))
    res = run_bass_kernel_spmd(
        nc, in_maps, core_ids=list(range(N_CORES)), trace=trace)
    LAST_EXEC_TIME_NS = res.exec_time_ns

    outs = np.stack([r["out"] for r in res.results])  # [8, BPC, 64, N]
    out = outs.reshape(B, D_V, N).transpose(0, 2, 1)  # [B, N, 64]
    return np.ascontiguousarray(out).astype(np.float32)


# revision 11
# speedup vs baseline: 1.2104x; 1.2104x over previous
"""Single-head attention (B=16, N=2048, d_in=256, d_qk=d_v=64) on 8 TRN2
NeuronCores, data-parallel over batch (2 batches per core, no collectives).

Math per batch b:
  q = x@wq + bq ; k = x@wk + bk ; v = x@wv + bv
  out = softmax(q k^T / 8) v

Device layout choices:
  - host feeds x^T (d on partitions) so every matmul has its contraction
    dim on partitions with zero on-device transposes
  - scores are computed TRANSPOSED: ST[m(keys) partitions, n(queries) free]
    so that P = exp(ST) is directly the rhs of the attention@V matmul
    (lhsT = V[m, dv]); exp needs no max-subtraction (scores sigma~0.6)
  - score matmuls have K=64 so PAIRS of key-chunks are row-packed into the
    two halves of the 128x128 PE array (tile_position via base partition);
    Q^T/K^T are duplicated into both partition halves to support this
  - software pipeline: engines execute in emission order, so the
    attention@V matmuls of query-half h-1 (plus next batch's projections)
    are emitted as FILLER between half h's score/exp pairs — the PE stays
    dense while ScalarE (the exp pacer) is never starved
  - wv is augmented with a 65th column of ones (via the bias row) so the
    softmax denominator appears as row 64 of the output accumulator
  - denominator reciprocal is broadcast across partitions with a ones
    matmul; final output is written as out^T [dv, n] and transposed on host
"""

import os
from collections import deque
from contextlib import ExitStack

import numpy as np

N_CORES = 8
B, N, D_IN, D_QK, D_V = 16, 2048, 256, 64, 64
BPC = B // N_CORES  # batches per core
EV1 = D_V + 1  # v augmented with ones column (softmax denominator)
NH = 1024  # query-dim width of an ot psum tile (2 banks)

_CACHE = {}

# exec time of the most recent profiled run (test harness convenience)
LAST_EXEC_TIME_NS = None


def _build_nc():
    import concourse.tile as tile
    from concourse import bacc, mybir

    f32 = mybir.dt.float32
    bf16 = mybir.dt.bfloat16
    Exp = mybir.ActivationFunctionType.Exp

    nc = bacc.Bacc(
        "TRN2", target_bir_lowering=False, debug=False,
        enable_asserts=True, num_devices=N_CORES,
    )

    xt = nc.dram_tensor("xt", [BPC, D_IN, N], bf16, kind="ExternalInput").ap()
    wq = nc.dram_tensor("wq", [D_IN, D_QK], bf16, kind="ExternalInput").ap()
    wk = nc.dram_tensor("wk", [D_IN, D_QK], bf16, kind="ExternalInput").ap()
    wv = nc.dram_tensor("wv", [D_IN, EV1], bf16, kind="ExternalInput").ap()
    bq = nc.dram_tensor("bq", [D_QK, 1], f32, kind="ExternalInput").ap()
    bk = nc.dram_tensor("bk", [D_QK, 1], f32, kind="ExternalInput").ap()
    bv = nc.dram_tensor("bv", [128, EV1], f32, kind="ExternalInput").ap()
    ones = nc.dram_tensor("ones", [1, D_V], bf16, kind="ExternalInput").ap()
    out = nc.dram_tensor("out", [BPC, D_V, N], f32, kind="ExternalOutput").ap()

    with tile.TileContext(nc) as tc, ExitStack() as ctx:
        consts = ctx.enter_context(tc.tile_pool(name="consts", bufs=1))
        xt_pool = ctx.enter_context(tc.tile_pool(name="xt", bufs=2))
        qk_pool = ctx.enter_context(tc.tile_pool(name="qk", bufs=2))
        v_pool = ctx.enter_context(tc.tile_pool(name="v", bufs=2))
        p_pool = ctx.enter_context(tc.tile_pool(name="p", bufs=34))
        rb_pool = ctx.enter_context(tc.tile_pool(name="rb", bufs=2))
        small = ctx.enter_context(tc.tile_pool(name="small", bufs=3))
        outp = ctx.enter_context(tc.tile_pool(name="outp", bufs=3))
        # one shared PSUM pool: 4 slots x 2 banks = all 8 banks
        psum = ctx.enter_context(tc.tile_pool(name="psum", bufs=4, space="PSUM"))

        lp = nc.allow_low_precision(reason="bf16 attention intermediates")
        lp.__enter__()

        # ---- constants (loaded once) ----
        wq_sb = consts.tile([128, 2 * D_QK], bf16, tag="wq")
        wk_sb = consts.tile([128, 2 * D_QK], bf16, tag="wk")
        wv_sb = consts.tile([128, 2 * EV1], bf16, tag="wv")
        for kk in range(2):
            nc.sync.dma_start(
                wq_sb[:, kk * D_QK:(kk + 1) * D_QK], wq[kk * 128:(kk + 1) * 128, :])
            nc.sync.dma_start(
                wk_sb[:, kk * D_QK:(kk + 1) * D_QK], wk[kk * 128:(kk + 1) * 128, :])
            nc.sync.dma_start(
                wv_sb[:, kk * EV1:(kk + 1) * EV1], wv[kk * 128:(kk + 1) * 128, :])
        bq_sb = consts.tile([D_QK, 1], f32, tag="bq")
        bk_sb = consts.tile([D_QK, 1], f32, tag="bk")
        bv_sb = consts.tile([128, EV1], f32, tag="bv")
        ones_sb = consts.tile([1, D_V], bf16, tag="ones")
        nc.sync.dma_start(bq_sb[:], bq[:, :])
        nc.sync.dma_start(bk_sb[:], bk[:, :])
        nc.sync.dma_start(bv_sb[:], bv[:, :])
        nc.sync.dma_start(ones_sb[:], ones[:, :])
        # warm the exp table during the initial DMAs
        warm = small.tile([1, 1], f32, tag="warm")
        nc.scalar.activation(warm[:], bq_sb[0:1, 0:1], Exp)

        # per-batch SBUF tiles, allocated lazily
        xt_sb = [None] * BPC
        qtd = [None] * BPC
        ktd = [None] * BPC
        v_sb = [None] * BPC

        def emit_xt_dma(b):
            xt_sb[b] = xt_pool.tile([128, 2 * N], bf16, tag="xt", name=f"xtsb{b}")
            for kk in range(2):
                for hh in range(2):
                    nc.sync.dma_start(
                        xt_sb[b][:, kk * N + hh * NH: kk * N + (hh + 1) * NH],
                        xt[b, kk * 128:(kk + 1) * 128, hh * NH:(hh + 1) * NH])

        def qkv_units(b):
            """Projection work as small PE units (closures)."""
            qtd[b] = qk_pool.tile([128, N], bf16, tag="qt", name=f"qtd{b}")
            ktd[b] = qk_pool.tile([128, N], bf16, tag="kt", name=f"ktd{b}")
            v_sb[b] = v_pool.tile([128, 16 * EV1], bf16, tag="v", name=f"vsb{b}")
            units = []

            def qk_unit(w_sb, b_sb, dst, jh):
                def emit():
                    ps = psum.tile([D_QK, NH], f32, tag="big")
                    for kk in range(2):
                        for j in range(2):
                            nc.tensor.matmul(
                                ps[:, j * 512:(j + 1) * 512],
                                w_sb[:, kk * D_QK:(kk + 1) * D_QK],
                                xt_sb[b][:, kk * N + jh * NH + j * 512:
                                          kk * N + jh * NH + (j + 1) * 512],
                                start=(kk == 0), stop=(kk == 1))
                    for j in range(2):
                        jj = jh * NH + j * 512
                        nc.vector.tensor_scalar_add(
                            dst[0:D_QK, jj:jj + 512],
                            ps[:, j * 512:(j + 1) * 512], b_sb[:])
                        # duplicate into partitions 64..127 (row-packing)
                        nc.sync.dma_start(
                            dst[D_QK:128, jj:jj + 512], dst[0:D_QK, jj:jj + 512])
                return emit

            def v_unit(m_lo):
                def emit():
                    for m in range(m_lo, m_lo + 4):
                        ps = psum.tile([128, EV1], f32, tag="big")
                        for kk in range(2):
                            nc.tensor.matmul(
                                ps[:],
                                xt_sb[b][:, kk * N + m * 128: kk * N + (m + 1) * 128],
                                wv_sb[:, kk * EV1:(kk + 1) * EV1],
                                start=(kk == 0), stop=(kk == 1))
                        nc.vector.tensor_add(
                            v_sb[b][:, m * EV1:(m + 1) * EV1], ps[:], bv_sb[:])
                return emit

            for jh in range(2):
                units.append(qk_unit(wq_sb, bq_sb, qtd[b], jh))
                units.append(qk_unit(wk_sb, bk_sb, ktd[b], jh))
            for m_lo in range(0, 16, 4):
                units.append(v_unit(m_lo))
            return units

        def ot_units(b, h, p_tiles):
            """Attention@V for (b, h): 8 matmul units + 1 epilogue unit."""
            ot = psum.tile([EV1, NH], f32, tag="big")
            units = []

            def mm_unit(mp):
                def emit():
                    m0, m1 = 2 * mp, 2 * mp + 1
                    p0, p1 = p_tiles[mp]
                    nc.tensor.matmul(
                        ot[:, 0:512], v_sb[b][:, m0 * EV1:(m0 + 1) * EV1],
                        p0[:, 0:512],
                        start=(mp == 0), stop=False, skip_group_check=True)
                    nc.tensor.matmul(
                        ot[:, 512:1024], v_sb[b][:, m0 * EV1:(m0 + 1) * EV1],
                        p1[:, 0:512],
                        start=(mp == 0), stop=False, skip_group_check=True)
                    nc.tensor.matmul(
                        ot[:, 0:512], v_sb[b][:, m1 * EV1:(m1 + 1) * EV1],
                        p0[:, 512:1024],
                        start=False, stop=(mp == 7), skip_group_check=True)
                    nc.tensor.matmul(
                        ot[:, 512:1024], v_sb[b][:, m1 * EV1:(m1 + 1) * EV1],
                        p1[:, 512:1024],
                        start=False, stop=(mp == 7), skip_group_check=True)
                return emit

            def epilogue():
                den_sb = small.tile([1, NH], f32, tag="den")
                nc.vector.tensor_copy(den_sb[:], ot[D_V:EV1, :])
                rcp = small.tile([1, NH], f32, tag="rcp")
                nc.vector.reciprocal_approx_fast(rcp[:], den_sb[:])
                rcp16 = small.tile([1, NH], bf16, tag="rcp16")
                nc.vector.tensor_copy(rcp16[:], rcp[:])
                rb_sb = rb_pool.tile([D_V, NH], bf16, tag="rb")
                for j in range(NH // 512):
                    rb_ps = psum.tile([D_V, 512], f32, tag="big")
                    nc.tensor.matmul(
                        rb_ps[:], ones_sb[:],
                        rcp16[:, j * 512:(j + 1) * 512],
                        start=True, stop=True)
                    nc.vector.tensor_copy(rb_sb[:, j * 512:(j + 1) * 512], rb_ps[:])
                o_sb = outp.tile([D_V, NH], f32, tag="o")
                nc.vector.tensor_mul(o_sb[:], ot[0:D_V, :], rb_sb[:])
                nc.sync.dma_start(out[b, :, h * NH:(h + 1) * NH], o_sb[:])

            for mp in range(8):
                units.append(mm_unit(mp))
            units.append(epilogue)
            return units

        filler = deque()  # PE work to interleave into score/exp emission
        must_emit = deque()  # work the NEXT phase A depends on (drained first)

        def pop_filler(k):
            for _ in range(k):
                if must_emit:
                    must_emit.popleft()()
                elif filler:
                    filler.popleft()()
                else:
                    break

        # ---- prologue: batch 0 inputs + projections (nothing to overlap) ----
        emit_xt_dma(0)
        if BPC > 1:
            emit_xt_dma(1)
        for u in qkv_units(0):
            u()
        if BPC > 1:
            must_emit.extend(qkv_units(1))

        # ---- attention: phase A per half, previous half's work as filler ----
        for b in range(BPC):
            for h in range(N // NH):
                p_tiles = []
                for mp in range(8):  # key-chunk pair (2*mp, 2*mp+1)
                    m0, m1 = 2 * mp, 2 * mp + 1
                    pjs = []
                    for js in range(2):  # 512-wide query slice in half
                        q0 = h * NH + js * 512
                        st = psum.tile([128, NH], f32, tag="big")
                        # row-packed pair: array rows 0-63 and 64-127
                        nc.tensor.matmul(
                            st[:, 0:512],
                            ktd[b][0:D_QK, m0 * 128:(m0 + 1) * 128],
                            qtd[b][0:D_QK, q0:q0 + 512],
                            start=True, stop=True)
                        nc.tensor.matmul(
                            st[:, 512:1024],
                            ktd[b][D_QK:128, m1 * 128:(m1 + 1) * 128],
                            qtd[b][D_QK:128, q0:q0 + 512],
                            start=True, stop=True)
                        p = p_pool.tile([128, NH], bf16, tag="p")
                        nc.scalar.activation(p[:], st[:], Exp)
                        pjs.append(p)
                    p_tiles.append(pjs)
                    pop_filler(2)
                # everything the next phase A needs must be emitted by now
                while must_emit:
                    must_emit.popleft()()
                filler.extend(ot_units(b, h, p_tiles))

        while must_emit:
            must_emit.popleft()()
        while filler:
            filler.popleft()()

        lp.__exit__(None, None, None)

    nc.compile()
    return nc


def _get_nc():
    if "nc" not in _CACHE:
        _CACHE["nc"] = _build_nc()
    return _CACHE["nc"]


def kernel(x, wq, bq, wk, bk, wv, bv):
    global LAST_EXEC_TIME_NS
    import ml_dtypes
    from concourse.bass_utils import run_bass_kernel_spmd

    bf16 = ml_dtypes.bfloat16
    x = np.asarray(x, dtype=np.float32)
    wq = np.asarray(wq, dtype=np.float32)
    wk = np.asarray(wk, dtype=np.float32)
    wv = np.asarray(wv, dtype=np.float32)
    bq = np.asarray(bq, dtype=np.float32)
    bk = np.asarray(bk, dtype=np.float32)
    bv = np.asarray(bv, dtype=np.float32)

    # host-side layout/precision prep
    xt = np.ascontiguousarray(x.transpose(0, 2, 1)).astype(bf16)  # [B, D, N]
    wq8 = (wq / 8.0).astype(bf16)  # fold 1/sqrt(d_qk) into q projection
    bq8 = (bq / 8.0).astype(np.float32).reshape(D_QK, 1)
    wkb = wk.astype(bf16)
    bkb = bk.astype(np.float32).reshape(D_QK, 1)
    wv_aug = np.zeros((D_IN, EV1), np.float32)
    wv_aug[:, :D_V] = wv
    wv_augb = wv_aug.astype(bf16)
    bv_aug = np.zeros((128, EV1), np.float32)
    bv_aug[:, :D_V] = bv  # broadcast bias to all partitions
    bv_aug[:, D_V] = 1.0  # ones column -> softmax denominator
    ones = np.ones((1, D_V), bf16)

    in_maps = []
    for c in range(N_CORES):
        in_maps.append({
            "xt": np.ascontiguousarray(xt[BPC * c: BPC * (c + 1)]),
            "wq": wq8, "wk": wkb, "wv": wv_augb,
            "bq": bq8, "bk": bkb, "bv": bv_aug, "ones": ones,
        })

    nc = _get_nc()
    trace = bool(int(os.environ.get("ATTN_PROFILE", "0")))
    res = run_bass_kernel_spmd(
        nc, in_maps, core_ids=list(range(N_CORES)), trace=trace)
    LAST_EXEC_TIME_NS = res.exec_time_ns

    outs = np.stack([r["out"] for r in res.results])  # [8, BPC, 64, N]
    out = outs.reshape(B, D_V, N).transpose(0, 2, 1)  # [B, N, 64]
    return np.ascontiguousarray(out).astype(np.float32)


# revision 13
# speedup vs baseline: 1.2706x; 1.0497x over previous
"""Single-head attention (B=16, N=2048, d_in=256, d_qk=d_v=64) on 8 TRN2
NeuronCores, data-parallel over batch (2 batches per core, no collectives).

Math per batch b:
  q = x@wq + bq ; k = x@wk + bk ; v = x@wv + bv
  out = softmax(q k^T / 8) v

Device layout choices:
  - host feeds x^T (d on partitions) so every matmul has its contraction
    dim on partitions with zero on-device transposes
  - scores are computed TRANSPOSED: ST[m(keys) partitions, n(queries) free]
    so that P = exp(ST) is directly the rhs of the attention@V matmul
    (lhsT = V[m, dv]); exp needs no max-subtraction (scores sigma~0.6)
  - score matmuls have K=64 so PAIRS of key-chunks are row-packed into the
    two halves of the 128x128 PE array (tile_position via base partition);
    Q^T/K^T are duplicated into both partition halves to support this
  - software pipeline: engines execute in emission order, so the
    attention@V matmuls of query-half h-1 (plus next batch's projections)
    are emitted as FILLER between half h's score/exp pairs — the PE stays
    dense while ScalarE (the exp pacer) is never starved
  - wv is augmented with a 65th column of ones (via the bias row) so the
    softmax denominator appears as row 64 of the output accumulator
  - denominator reciprocal is broadcast across partitions with a ones
    matmul; final output is written as out^T [dv, n] and transposed on host
"""

import os
from collections import deque
from contextlib import ExitStack

import numpy as np

N_CORES = 8
B, N, D_IN, D_QK, D_V = 16, 2048, 256, 64, 64
BPC = B // N_CORES  # batches per core
EV1 = D_V + 1  # v augmented with ones column (softmax denominator)
NH = 1024  # query-dim width of an ot psum tile (2 banks)

_CACHE = {}

# exec time of the most recent profiled run (test harness convenience)
LAST_EXEC_TIME_NS = None


def _build_nc():
    import concourse.tile as tile
    from concourse import bacc, mybir

    f32 = mybir.dt.float32
    bf16 = mybir.dt.bfloat16
    Exp = mybir.ActivationFunctionType.Exp

    nc = bacc.Bacc(
        "TRN2", target_bir_lowering=False, debug=False,
        enable_asserts=True, num_devices=N_CORES,
    )

    xt = nc.dram_tensor("xt", [BPC, D_IN, N], bf16, kind="ExternalInput").ap()
    wq = nc.dram_tensor("wq", [D_IN, D_QK], bf16, kind="ExternalInput").ap()
    wk = nc.dram_tensor("wk", [D_IN, D_QK], bf16, kind="ExternalInput").ap()
    wv = nc.dram_tensor("wv", [D_IN, EV1], bf16, kind="ExternalInput").ap()
    bq = nc.dram_tensor("bq", [D_QK, 1], f32, kind="ExternalInput").ap()
    bk = nc.dram_tensor("bk", [D_QK, 1], f32, kind="ExternalInput").ap()
    bv = nc.dram_tensor("bv", [128, EV1], f32, kind="ExternalInput").ap()
    ones = nc.dram_tensor("ones", [1, D_V], bf16, kind="ExternalInput").ap()
    out = nc.dram_tensor("out", [BPC, D_V, N], f32, kind="ExternalOutput").ap()

    with tile.TileContext(nc) as tc, ExitStack() as ctx:
        consts = ctx.enter_context(tc.tile_pool(name="consts", bufs=1))
        xt_pool = ctx.enter_context(tc.tile_pool(name="xt", bufs=2))
        qk_pool = ctx.enter_context(tc.tile_pool(name="qk", bufs=2))
        v_pool = ctx.enter_context(tc.tile_pool(name="v", bufs=2))
        p_pool = ctx.enter_context(tc.tile_pool(name="p", bufs=34))
        rb_pool = ctx.enter_context(tc.tile_pool(name="rb", bufs=2))
        small = ctx.enter_context(tc.tile_pool(name="small", bufs=3))
        outp = ctx.enter_context(tc.tile_pool(name="outp", bufs=3))
        # one shared PSUM pool: 4 slots x 2 banks = all 8 banks
        psum = ctx.enter_context(tc.tile_pool(name="psum", bufs=4, space="PSUM"))

        lp = nc.allow_low_precision(reason="bf16 attention intermediates")
        lp.__enter__()

        # ---- constants (loaded once) ----
        wq_sb = consts.tile([128, 2 * D_QK], bf16, tag="wq")
        wk_sb = consts.tile([128, 2 * D_QK], bf16, tag="wk")
        wv_sb = consts.tile([128, 2 * EV1], bf16, tag="wv")
        for kk in range(2):
            nc.sync.dma_start(
                wq_sb[:, kk * D_QK:(kk + 1) * D_QK], wq[kk * 128:(kk + 1) * 128, :])
            nc.sync.dma_start(
                wk_sb[:, kk * D_QK:(kk + 1) * D_QK], wk[kk * 128:(kk + 1) * 128, :])
            nc.sync.dma_start(
                wv_sb[:, kk * EV1:(kk + 1) * EV1], wv[kk * 128:(kk + 1) * 128, :])
        bq_sb = consts.tile([D_QK, 1], f32, tag="bq")
        bk_sb = consts.tile([D_QK, 1], f32, tag="bk")
        bv_sb = consts.tile([128, EV1], f32, tag="bv")
        ones_sb = consts.tile([1, D_V], bf16, tag="ones")
        nc.sync.dma_start(bq_sb[:], bq[:, :])
        nc.sync.dma_start(bk_sb[:], bk[:, :])
        nc.sync.dma_start(bv_sb[:], bv[:, :])
        nc.sync.dma_start(ones_sb[:], ones[:, :])
        # warm the exp table during the initial DMAs
        warm = small.tile([1, 1], f32, tag="warm")
        nc.scalar.activation(warm[:], bq_sb[0:1, 0:1], Exp)

        # per-batch SBUF tiles, allocated lazily
        xt_sb = [None] * BPC
        qtd = [None] * BPC
        ktd = [None] * BPC
        v_sb = [None] * BPC

        def emit_xt_dma(b):
            xt_sb[b] = xt_pool.tile([128, 2 * N], bf16, tag="xt", name=f"xtsb{b}")
            for kk in range(2):
                for hh in range(2):
                    nc.sync.dma_start(
                        xt_sb[b][:, kk * N + hh * NH: kk * N + (hh + 1) * NH],
                        xt[b, kk * 128:(kk + 1) * 128, hh * NH:(hh + 1) * NH])

        def qkv_units(b):
            """Projection work as small PE units (closures)."""
            qtd[b] = qk_pool.tile([128, N], bf16, tag="qt", name=f"qtd{b}")
            ktd[b] = qk_pool.tile([128, N], bf16, tag="kt", name=f"ktd{b}")
            v_sb[b] = v_pool.tile([128, 16 * EV1], bf16, tag="v", name=f"vsb{b}")
            units = []

            def qk_unit(w_sb, b_sb, dst, jh):
                def emit():
                    ps = psum.tile([D_QK, NH], f32, tag="big")
                    for kk in range(2):
                        for j in range(2):
                            nc.tensor.matmul(
                                ps[:, j * 512:(j + 1) * 512],
                                w_sb[:, kk * D_QK:(kk + 1) * D_QK],
                                xt_sb[b][:, kk * N + jh * NH + j * 512:
                                          kk * N + jh * NH + (j + 1) * 512],
                                start=(kk == 0), stop=(kk == 1))
                    for j in range(2):
                        jj = jh * NH + j * 512
                        nc.vector.tensor_scalar_add(
                            dst[0:D_QK, jj:jj + 512],
                            ps[:, j * 512:(j + 1) * 512], b_sb[:])
                        # duplicate into partitions 64..127 (row-packing)
                        nc.sync.dma_start(
                            dst[D_QK:128, jj:jj + 512], dst[0:D_QK, jj:jj + 512])
                return emit

            def v_unit(m_lo):
                def emit():
                    for m in range(m_lo, m_lo + 4):
                        ps = psum.tile([128, EV1], f32, tag="big")
                        for kk in range(2):
                            nc.tensor.matmul(
                                ps[:],
                                xt_sb[b][:, kk * N + m * 128: kk * N + (m + 1) * 128],
                                wv_sb[:, kk * EV1:(kk + 1) * EV1],
                                start=(kk == 0), stop=(kk == 1))
                        nc.vector.tensor_add(
                            v_sb[b][:, m * EV1:(m + 1) * EV1], ps[:], bv_sb[:])
                return emit

            for jh in range(2):
                units.append(qk_unit(wq_sb, bq_sb, qtd[b], jh))
                units.append(qk_unit(wk_sb, bk_sb, ktd[b], jh))
            for m_lo in range(0, 16, 4):
                units.append(v_unit(m_lo))
            return units

        def emit_ot_mm(b, ot, mp, pjs):
            """One attention@V accumulation unit (4 matmuls, weight-shared)."""
            m0, m1 = 2 * mp, 2 * mp + 1
            p0, p1 = pjs
            nc.tensor.matmul(
                ot[:, 0:512], v_sb[b][:, m0 * EV1:(m0 + 1) * EV1],
                p0[:, 0:512],
                start=(mp == 0), stop=False, skip_group_check=True)
            nc.tensor.matmul(
                ot[:, 512:1024], v_sb[b][:, m0 * EV1:(m0 + 1) * EV1],
                p1[:, 0:512],
                start=(mp == 0), stop=False, skip_group_check=True)
            nc.tensor.matmul(
                ot[:, 0:512], v_sb[b][:, m1 * EV1:(m1 + 1) * EV1],
                p0[:, 512:1024],
                start=False, stop=(mp == 7), skip_group_check=True)
            nc.tensor.matmul(
                ot[:, 512:1024], v_sb[b][:, m1 * EV1:(m1 + 1) * EV1],
                p1[:, 512:1024],
                start=False, stop=(mp == 7), skip_group_check=True)

        def emit_epilogue(b, h, ot):
            den_sb = small.tile([1, NH], f32, tag="den")
            nc.vector.tensor_copy(den_sb[:], ot[D_V:EV1, :])
            rcp = small.tile([1, NH], f32, tag="rcp")
            nc.vector.reciprocal_approx_fast(rcp[:], den_sb[:])
            rcp16 = small.tile([1, NH], bf16, tag="rcp16")
            nc.vector.tensor_copy(rcp16[:], rcp[:])
            rb_sb = rb_pool.tile([D_V, NH], bf16, tag="rb")
            for j in range(NH // 512):
                rb_ps = psum.tile([D_V, 512], f32, tag="big")
                nc.tensor.matmul(
                    rb_ps[:], ones_sb[:],
                    rcp16[:, j * 512:(j + 1) * 512],
                    start=True, stop=True)
                nc.vector.tensor_copy(rb_sb[:, j * 512:(j + 1) * 512], rb_ps[:])
            o_sb = outp.tile([D_V, NH], f32, tag="o")
            nc.vector.tensor_mul(o_sb[:], ot[0:D_V, :], rb_sb[:])
            nc.sync.dma_start(out[b, :, h * NH:(h + 1) * NH], o_sb[:])

        filler = deque()  # PE work to interleave into score/exp emission
        must_emit = deque()  # upcoming-batch QKV work (drained with priority)

        def pop_filler(k):
            for _ in range(k):
                if must_emit:
                    must_emit.popleft()()
                elif filler:
                    filler.popleft()()
                else:
                    break

        # ---- prologue: inputs + minimal projections before first scores ----
        emit_xt_dma(0)
        if BPC > 1:
            emit_xt_dma(1)
        u0 = qkv_units(0)
        u0[0]()  # Q columns 0..1023 (query half 0)
        u0[1]()  # K columns 0..1023 (key chunks 0..7)
        must_emit.extend(u0[2:])  # K hi, Q hi, V units — popped during half 0
        if BPC > 1:
            must_emit.extend(qkv_units(1))

        # ---- attention: exp-paced pipeline; attention@V lags by 2 pairs ----
        OT_LAG = 2
        for b in range(BPC):
            # this batch's projections must precede its scores on the PE
            while must_emit:
                must_emit.popleft()()
            for h in range(N // NH):
                ot = psum.tile([EV1, NH], f32, tag="big", name=f"ot{b}_{h}")
                p_tiles = []
                for mp in range(8):  # key-chunk pair (2*mp, 2*mp+1)
                    m0, m1 = 2 * mp, 2 * mp + 1
                    pjs = []
                    for js in range(2):  # 512-wide query slice in half
                        q0 = h * NH + js * 512
                        st = psum.tile([128, NH], f32, tag="big", name="st")
                        # row-packed pair: array rows 0-63 and 64-127
                        nc.tensor.matmul(
                            st[:, 0:512],
                            ktd[b][0:D_QK, m0 * 128:(m0 + 1) * 128],
                            qtd[b][0:D_QK, q0:q0 + 512],
                            start=True, stop=True)
                        nc.tensor.matmul(
                            st[:, 512:1024],
                            ktd[b][D_QK:128, m1 * 128:(m1 + 1) * 128],
                            qtd[b][D_QK:128, q0:q0 + 512],
                            start=True, stop=True)
                        p = p_pool.tile([128, NH], bf16, tag="p", name="p")
                        nc.scalar.activation(p[:], st[:], Exp)
                        pjs.append(p)
                    p_tiles.append(pjs)
                    if mp >= OT_LAG:
                        emit_ot_mm(b, ot, mp - OT_LAG, p_tiles[mp - OT_LAG])
                        pop_filler(1)
                    else:
                        pop_filler(2)
                # defer the last OT_LAG units + epilogue into the next half
                for mp_tail in range(8 - OT_LAG, 8):
                    filler.append(
                        (lambda bb=b, oo=ot, mm=mp_tail, pp=p_tiles[mp_tail]:
                         emit_ot_mm(bb, oo, mm, pp)))
                filler.append(lambda bb=b, hh=h, oo=ot: emit_epilogue(bb, hh, oo))

        while filler:
            filler.popleft()()

        lp.__exit__(None, None, None)

    nc.compile()
    return nc


def _get_nc():
    if "nc" not in _CACHE:
        _CACHE["nc"] = _build_nc()
    return _CACHE["nc"]


def kernel(x, wq, bq, wk, bk, wv, bv):
    global LAST_EXEC_TIME_NS
    import ml_dtypes
    from concourse.bass_utils import run_bass_kernel_spmd

    bf16 = ml_dtypes.bfloat16
    x = np.asarray(x, dtype=np.float32)
    wq = np.asarray(wq, dtype=np.float32)
    wk = np.asarray(wk, dtype=np.float32)
    wv = np.asarray(wv, dtype=np.float32)
    bq = np.asarray(bq, dtype=np.float32)
    bk = np.asarray(bk, dtype=np.float32)
    bv = np.asarray(bv, dtype=np.float32)

    # host-side layout/precision prep
    xt = np.ascontiguousarray(x.transpose(0, 2, 1)).astype(bf16)  # [B, D, N]
    wq8 = (wq / 8.0).astype(bf16)  # fold 1/sqrt(d_qk) into q projection
    bq8 = (bq / 8.0).astype(np.float32).reshape(D_QK, 1)
    wkb = wk.astype(bf16)
    bkb = bk.astype(np.float32).reshape(D_QK, 1)
    wv_aug = np.zeros((D_IN, EV1), np.float32)
    wv_aug[:, :D_V] = wv
    wv_augb = wv_aug.astype(bf16)
    bv_aug = np.zeros((128, EV1), np.float32)
    bv_aug[:, :D_V] = bv  # broadcast bias to all partitions
    bv_aug[:, D_V] = 1.0  # ones column -> softmax denominator
    ones = np.ones((1, D_V), bf16)

    in_maps = []
    for c in range(N_CORES):
        in_maps.append({
            "xt": np.ascontiguousarray(xt[BPC * c: BPC * (c + 1)]),
            "wq": wq8, "wk": wkb, "wv": wv_augb,
            "bq": bq8, "bk": bkb, "bv": bv_aug, "ones": ones,
        })

    nc = _get_nc()
    trace = bool(int(os.environ.get("ATTN_PROFILE", "0")))
    res = run_bass_kernel_spmd(
        nc, in_maps, core_ids=list(range(N_CORES)), trace=trace)
    LAST_EXEC_TIME_NS = res.exec_time_ns

    outs = np.stack([r["out"] for r in res.results])  # [8, BPC, 64, N]
    out = outs.reshape(B, D_V, N).transpose(0, 2, 1)  # [B, N, 64]
    return np.ascontiguousarray(out).astype(np.float32)


# revision 14
# speedup vs baseline: 1.2852x; 1.0115x over previous
"""Single-head attention (B=16, N=2048, d_in=256, d_qk=d_v=64) on 8 TRN2
NeuronCores, data-parallel over batch (2 batches per core, no collectives).

Math per batch b:
  q = x@wq + bq ; k = x@wk + bk ; v = x@wv + bv
  out = softmax(q k^T / 8) v

Device layout choices:
  - host feeds x^T (d on partitions) so every matmul has its contraction
    dim on partitions with zero on-device transposes
  - scores are computed TRANSPOSED: ST[m(keys) partitions, n(queries) free]
    so that P = exp(ST) is directly the rhs of the attention@V matmul
    (lhsT = V[m, dv]); exp needs no max-subtraction (scores sigma~0.6)
  - score matmuls have K=64 so PAIRS of key-chunks are row-packed into the
    two halves of the 128x128 PE array (tile_position via base partition);
    Q^T/K^T are duplicated into both partition halves to support this
  - software pipeline: engines execute in emission order, so the
    attention@V matmuls of query-half h-1 (plus next batch's projections)
    are emitted as FILLER between half h's score/exp pairs — the PE stays
    dense while ScalarE (the exp pacer) is never starved
  - wv is augmented with a 65th column of ones (via the bias row) so the
    softmax denominator appears as row 64 of the output accumulator
  - denominator reciprocal is broadcast across partitions with a ones
    matmul; final output is written as out^T [dv, n] and transposed on host
"""

import os
from collections import deque
from contextlib import ExitStack

import numpy as np

N_CORES = 8
B, N, D_IN, D_QK, D_V = 16, 2048, 256, 64, 64
BPC = B // N_CORES  # batches per core
EV1 = D_V + 1  # v augmented with ones column (softmax denominator)
NH = 1024  # query-dim width of an ot psum tile (2 banks)

_CACHE = {}

# exec time of the most recent profiled run (test harness convenience)
LAST_EXEC_TIME_NS = None


def _build_nc():
    import concourse.tile as tile
    from concourse import bacc, mybir

    f32 = mybir.dt.float32
    bf16 = mybir.dt.bfloat16
    Exp = mybir.ActivationFunctionType.Exp

    nc = bacc.Bacc(
        "TRN2", target_bir_lowering=False, debug=False,
        enable_asserts=True, num_devices=N_CORES,
    )

    xt = nc.dram_tensor("xt", [BPC, D_IN, N], bf16, kind="ExternalInput").ap()
    wq = nc.dram_tensor("wq", [D_IN, D_QK], bf16, kind="ExternalInput").ap()
    wk = nc.dram_tensor("wk", [D_IN, D_QK], bf16, kind="ExternalInput").ap()
    wv = nc.dram_tensor("wv", [D_IN, EV1], bf16, kind="ExternalInput").ap()
    bq = nc.dram_tensor("bq", [D_QK, 1], f32, kind="ExternalInput").ap()
    bk = nc.dram_tensor("bk", [D_QK, 1], f32, kind="ExternalInput").ap()
    bv = nc.dram_tensor("bv", [128, EV1], f32, kind="ExternalInput").ap()
    ones = nc.dram_tensor("ones", [1, D_V], bf16, kind="ExternalInput").ap()
    out = nc.dram_tensor("out", [BPC, D_V, N], f32, kind="ExternalOutput").ap()

    with tile.TileContext(nc) as tc, ExitStack() as ctx:
        consts = ctx.enter_context(tc.tile_pool(name="consts", bufs=1))
        xt_pool = ctx.enter_context(tc.tile_pool(name="xt", bufs=2))
        qk_pool = ctx.enter_context(tc.tile_pool(name="qk", bufs=2))
        v_pool = ctx.enter_context(tc.tile_pool(name="v", bufs=2))
        p_pool = ctx.enter_context(tc.tile_pool(name="p", bufs=34))
        rb_pool = ctx.enter_context(tc.tile_pool(name="rb", bufs=2))
        small = ctx.enter_context(tc.tile_pool(name="small", bufs=3))
        outp = ctx.enter_context(tc.tile_pool(name="outp", bufs=3))
        # one shared PSUM pool: 4 slots x 2 banks = all 8 banks
        psum = ctx.enter_context(tc.tile_pool(name="psum", bufs=4, space="PSUM"))

        lp = nc.allow_low_precision(reason="bf16 attention intermediates")
        lp.__enter__()

        # ---- constants (loaded once) ----
        wq_sb = consts.tile([128, 2 * D_QK], bf16, tag="wq")
        wk_sb = consts.tile([128, 2 * D_QK], bf16, tag="wk")
        wv_sb = consts.tile([128, 2 * EV1], bf16, tag="wv")
        for kk in range(2):
            nc.sync.dma_start(
                wq_sb[:, kk * D_QK:(kk + 1) * D_QK], wq[kk * 128:(kk + 1) * 128, :])
            nc.sync.dma_start(
                wk_sb[:, kk * D_QK:(kk + 1) * D_QK], wk[kk * 128:(kk + 1) * 128, :])
            nc.sync.dma_start(
                wv_sb[:, kk * EV1:(kk + 1) * EV1], wv[kk * 128:(kk + 1) * 128, :])
        bq_sb = consts.tile([D_QK, 1], f32, tag="bq")
        bk_sb = consts.tile([D_QK, 1], f32, tag="bk")
        bv_sb = consts.tile([128, EV1], f32, tag="bv")
        ones_sb = consts.tile([1, D_V], bf16, tag="ones")
        nc.sync.dma_start(bq_sb[:], bq[:, :])
        nc.sync.dma_start(bk_sb[:], bk[:, :])
        nc.sync.dma_start(bv_sb[:], bv[:, :])
        nc.sync.dma_start(ones_sb[:], ones[:, :])
        # warm the exp table during the initial DMAs
        warm = small.tile([1, 1], f32, tag="warm")
        nc.scalar.activation(warm[:], bq_sb[0:1, 0:1], Exp)

        # per-batch SBUF tiles, allocated lazily
        xt_sb = [None] * BPC
        qtd = [None] * BPC
        ktd = [None] * BPC
        v_sb = [None] * BPC

        def emit_xt_dma(b):
            xt_sb[b] = xt_pool.tile([128, 2 * N], bf16, tag="xt", name=f"xtsb{b}")
            for kk in range(2):
                for hh in range(2):
                    nc.sync.dma_start(
                        xt_sb[b][:, kk * N + hh * NH: kk * N + (hh + 1) * NH],
                        xt[b, kk * 128:(kk + 1) * 128, hh * NH:(hh + 1) * NH])

        def qkv_units(b):
            """Projection work as small PE units (closures)."""
            qtd[b] = qk_pool.tile([128, N], bf16, tag="qt", name=f"qtd{b}")
            ktd[b] = qk_pool.tile([128, N], bf16, tag="kt", name=f"ktd{b}")
            v_sb[b] = v_pool.tile([128, 16 * EV1], bf16, tag="v", name=f"vsb{b}")
            units = []

            def qk_unit(w_sb, b_sb, dst, jh):
                def emit():
                    ps = psum.tile([D_QK, NH], f32, tag="big")
                    for kk in range(2):
                        for j in range(2):
                            nc.tensor.matmul(
                                ps[:, j * 512:(j + 1) * 512],
                                w_sb[:, kk * D_QK:(kk + 1) * D_QK],
                                xt_sb[b][:, kk * N + jh * NH + j * 512:
                                          kk * N + jh * NH + (j + 1) * 512],
                                start=(kk == 0), stop=(kk == 1))
                    for j in range(2):
                        jj = jh * NH + j * 512
                        nc.vector.tensor_scalar_add(
                            dst[0:D_QK, jj:jj + 512],
                            ps[:, j * 512:(j + 1) * 512], b_sb[:])
                        # duplicate into partitions 64..127 (row-packing)
                        nc.sync.dma_start(
                            dst[D_QK:128, jj:jj + 512], dst[0:D_QK, jj:jj + 512])
                return emit

            def v_unit(m_lo):
                def emit():
                    for m in range(m_lo, m_lo + 4):
                        ps = psum.tile([128, EV1], f32, tag="big")
                        for kk in range(2):
                            nc.tensor.matmul(
                                ps[:],
                                xt_sb[b][:, kk * N + m * 128: kk * N + (m + 1) * 128],
                                wv_sb[:, kk * EV1:(kk + 1) * EV1],
                                start=(kk == 0), stop=(kk == 1))
                        nc.vector.tensor_add(
                            v_sb[b][:, m * EV1:(m + 1) * EV1], ps[:], bv_sb[:])
                return emit

            for jh in range(2):
                units.append(qk_unit(wq_sb, bq_sb, qtd[b], jh))
                units.append(qk_unit(wk_sb, bk_sb, ktd[b], jh))
            for m_lo in range(0, 16, 4):
                units.append(v_unit(m_lo))
            return units

        def emit_ot_mm(b, ot, mp, pjs):
            """One attention@V accumulation unit (4 matmuls, weight-shared)."""
            m0, m1 = 2 * mp, 2 * mp + 1
            p0, p1 = pjs
            nc.tensor.matmul(
                ot[:, 0:512], v_sb[b][:, m0 * EV1:(m0 + 1) * EV1],
                p0[:, 0:512],
                start=(mp == 0), stop=False, skip_group_check=True)
            nc.tensor.matmul(
                ot[:, 512:1024], v_sb[b][:, m0 * EV1:(m0 + 1) * EV1],
                p1[:, 0:512],
                start=(mp == 0), stop=False, skip_group_check=True)
            nc.tensor.matmul(
                ot[:, 0:512], v_sb[b][:, m1 * EV1:(m1 + 1) * EV1],
                p0[:, 512:1024],
                start=False, stop=(mp == 7), skip_group_check=True)
            nc.tensor.matmul(
                ot[:, 512:1024], v_sb[b][:, m1 * EV1:(m1 + 1) * EV1],
                p1[:, 512:1024],
                start=False, stop=(mp == 7), skip_group_check=True)

        def emit_epilogue(b, h, ot):
            den_sb = small.tile([1, NH], f32, tag="den")
            nc.vector.tensor_copy(den_sb[:], ot[D_V:EV1, :])
            rcp = small.tile([1, NH], f32, tag="rcp")
            nc.vector.reciprocal_approx_fast(rcp[:], den_sb[:])
            rcp16 = small.tile([1, NH], bf16, tag="rcp16")
            nc.vector.tensor_copy(rcp16[:], rcp[:])
            rb_sb = rb_pool.tile([D_V, NH], bf16, tag="rb")
            for j in range(NH // 512):
                rb_ps = psum.tile([D_V, 512], f32, tag="big")
                nc.tensor.matmul(
                    rb_ps[:], ones_sb[:],
                    rcp16[:, j * 512:(j + 1) * 512],
                    start=True, stop=True)
                nc.vector.tensor_copy(rb_sb[:, j * 512:(j + 1) * 512], rb_ps[:])
            o_sb = outp.tile([D_V, NH], f32, tag="o")
            nc.vector.tensor_mul(o_sb[:], ot[0:D_V, :], rb_sb[:])
            nc.sync.dma_start(out[b, :, h * NH:(h + 1) * NH], o_sb[:])

        filler = deque()  # PE work to interleave into score/exp emission
        must_emit = deque()  # upcoming-batch QKV work (drained with priority)

        def pop_filler(k):
            for _ in range(k):
                if must_emit:
                    must_emit.popleft()()
                elif filler:
                    filler.popleft()()
                else:
                    break

        # ---- prologue: inputs + minimal projections before first scores ----
        emit_xt_dma(0)
        if BPC > 1:
            emit_xt_dma(1)
        u0 = qkv_units(0)
        u0[0]()  # Q columns 0..1023 (query half 0)
        u0[1]()  # K columns 0..1023 (key chunks 0..7)
        must_emit.extend(u0[2:])  # K hi, Q hi, V units — popped during half 0
        if BPC > 1:
            must_emit.extend(qkv_units(1))

        # ---- attention: exp-paced pipeline; attention@V lags by 2 pairs ----
        OT_LAG = 2
        for b in range(BPC):
            # this batch's projections must precede its scores on the PE
            # (batch 0's minimal prologue already ran; the rest is popped
            # as filler during half 0)
            while b > 0 and must_emit:
                must_emit.popleft()()
            for h in range(N // NH):
                ot = psum.tile([EV1, NH], f32, tag="big", name=f"ot{b}_{h}")
                p_tiles = []
                for mp in range(8):  # key-chunk pair (2*mp, 2*mp+1)
                    m0, m1 = 2 * mp, 2 * mp + 1
                    pjs = []
                    for js in range(2):  # 512-wide query slice in half
                        q0 = h * NH + js * 512
                        st = psum.tile([128, NH], f32, tag="big", name="st")
                        # row-packed pair: array rows 0-63 and 64-127
                        nc.tensor.matmul(
                            st[:, 0:512],
                            ktd[b][0:D_QK, m0 * 128:(m0 + 1) * 128],
                            qtd[b][0:D_QK, q0:q0 + 512],
                            start=True, stop=True)
                        nc.tensor.matmul(
                            st[:, 512:1024],
                            ktd[b][D_QK:128, m1 * 128:(m1 + 1) * 128],
                            qtd[b][D_QK:128, q0:q0 + 512],
                            start=True, stop=True)
                        p = p_pool.tile([128, NH], bf16, tag="p", name="p")
                        nc.scalar.activation(p[:], st[:], Exp)
                        pjs.append(p)
                    p_tiles.append(pjs)
                    if mp >= OT_LAG:
                        emit_ot_mm(b, ot, mp - OT_LAG, p_tiles[mp - OT_LAG])
                        pop_filler(1)
                    else:
                        pop_filler(2)
                # defer the last OT_LAG units + epilogue into the next half
                for mp_tail in range(8 - OT_LAG, 8):
                    filler.append(
                        (lambda bb=b, oo=ot, mm=mp_tail, pp=p_tiles[mp_tail]:
                         emit_ot_mm(bb, oo, mm, pp)))
                filler.append(lambda bb=b, hh=h, oo=ot: emit_epilogue(bb, hh, oo))

        while filler:
            filler.popleft()()

        lp.__exit__(None, None, None)

    nc.compile()
    return nc


def _get_nc():
    if "nc" not in _CACHE:
        _CACHE["nc"] = _build_nc()
    return _CACHE["nc"]


def kernel(x, wq, bq, wk, bk, wv, bv):
    global LAST_EXEC_TIME_NS
    import ml_dtypes
    from concourse.bass_utils import run_bass_kernel_spmd

    bf16 = ml_dtypes.bfloat16
    x = np.asarray(x, dtype=np.float32)
    wq = np.asarray(wq, dtype=np.float32)
    wk = np.asarray(wk, dtype=np.float32)
    wv = np.asarray(wv, dtype=np.float32)
    bq = np.asarray(bq, dtype=np.float32)
    bk = np.asarray(bk, dtype=np.float32)
    bv = np.asarray(bv, dtype=np.float32)

    # host-side layout/precision prep
    xt = np.ascontiguousarray(x.transpose(0, 2, 1)).astype(bf16)  # [B, D, N]
    wq8 = (wq / 8.0).astype(bf16)  # fold 1/sqrt(d_qk) into q projection
    bq8 = (bq / 8.0).astype(np.float32).reshape(D_QK, 1)
    wkb = wk.astype(bf16)
    bkb = bk.astype(np.float32).reshape(D_QK, 1)
    wv_aug = np.zeros((D_IN, EV1), np.float32)
    wv_aug[:, :D_V] = wv
    wv_augb = wv_aug.astype(bf16)
    bv_aug = np.zeros((128, EV1), np.float32)
    bv_aug[:, :D_V] = bv  # broadcast bias to all partitions
    bv_aug[:, D_V] = 1.0  # ones column -> softmax denominator
    ones = np.ones((1, D_V), bf16)

    in_maps = []
    for c in range(N_CORES):
        in_maps.append({
            "xt": np.ascontiguousarray(xt[BPC * c: BPC * (c + 1)]),
            "wq": wq8, "wk": wkb, "wv": wv_augb,
            "bq": bq8, "bk": bkb, "bv": bv_aug, "ones": ones,
        })

    nc = _get_nc()
    trace = bool(int(os.environ.get("ATTN_PROFILE", "0")))
    res = run_bass_kernel_spmd(
        nc, in_maps, core_ids=list(range(N_CORES)), trace=trace)
    LAST_EXEC_TIME_NS = res.exec_time_ns

    outs = np.stack([r["out"] for r in res.results])  # [8, BPC, 64, N]
    out = outs.reshape(B, D_V, N).transpose(0, 2, 1)  # [B, N, 64]
    return np.ascontiguousarray(out).astype(np.float32)


# revision 18
# speedup vs baseline: 1.3401x; 1.0427x over previous
"""Single-head attention (B=16, N=2048, d_in=256, d_qk=d_v=64) on 8 TRN2
NeuronCores, data-parallel over batch (2 batches per core, no collectives).

Math per batch b:
  q = x@wq + bq ; k = x@wk + bk ; v = x@wv + bv
  out = softmax(q k^T / 8) v

Device layout choices:
  - host feeds x^T (d on partitions) so every matmul has its contraction
    dim on partitions with zero on-device transposes
  - scores are computed TRANSPOSED: ST[m(keys) partitions, n(queries) free]
    so that P = exp(ST) is directly the rhs of the attention@V matmul
    (lhsT = V[m, dv]); exp needs no max-subtraction (scores sigma~0.6)
  - score matmuls have K=64 so PAIRS of key-chunks are row-packed into the
    two halves of the 128x128 PE array (tile_position via base partition);
    Q^T/K^T are duplicated into both partition halves to support this
  - software pipeline: engines execute in emission order, so the
    attention@V matmuls of query-half h-1 (plus next batch's projections)
    are emitted as FILLER between half h's score/exp pairs — the PE stays
    dense while ScalarE (the exp pacer) is never starved
  - wv is augmented with a 65th column of ones (via the bias row) so the
    softmax denominator appears as row 64 of the output accumulator
  - denominator reciprocal is broadcast across partitions with a ones
    matmul; final output is written as out^T [dv, n] and transposed on host
"""

import os
from collections import deque
from contextlib import ExitStack

import numpy as np

N_CORES = 8
B, N, D_IN, D_QK, D_V = 16, 2048, 256, 64, 64
BPC = B // N_CORES  # batches per core
EV1 = D_V + 1  # v augmented with ones column (softmax denominator)
NH = 1024  # query-dim width of an ot psum tile (2 banks)

_CACHE = {}

# exec time of the most recent profiled run (test harness convenience)
LAST_EXEC_TIME_NS = None


def _build_nc():
    import concourse.tile as tile
    from concourse import bacc, mybir

    f32 = mybir.dt.float32
    bf16 = mybir.dt.bfloat16
    Exp = mybir.ActivationFunctionType.Exp

    nc = bacc.Bacc(
        "TRN2", target_bir_lowering=False, debug=False,
        enable_asserts=True, num_devices=N_CORES,
    )

    xt = nc.dram_tensor("xt", [BPC, D_IN, N], bf16, kind="ExternalInput").ap()
    wq = nc.dram_tensor("wq", [D_IN, D_QK], bf16, kind="ExternalInput").ap()
    wk = nc.dram_tensor("wk", [D_IN, D_QK], bf16, kind="ExternalInput").ap()
    wv = nc.dram_tensor("wv", [D_IN, EV1], bf16, kind="ExternalInput").ap()
    bq = nc.dram_tensor("bq", [D_QK, 1], f32, kind="ExternalInput").ap()
    bk = nc.dram_tensor("bk", [D_QK, 1], f32, kind="ExternalInput").ap()
    bv = nc.dram_tensor("bv", [128, EV1], f32, kind="ExternalInput").ap()
    ones = nc.dram_tensor("ones", [1, D_V], bf16, kind="ExternalInput").ap()
    out = nc.dram_tensor("out", [BPC, D_V, N], f32, kind="ExternalOutput").ap()

    with tile.TileContext(nc) as tc, ExitStack() as ctx:
        consts = ctx.enter_context(tc.tile_pool(name="consts", bufs=1))
        xt_pool = ctx.enter_context(tc.tile_pool(name="xt", bufs=2))
        qk_pool = ctx.enter_context(tc.tile_pool(name="qk", bufs=2))
        v_pool = ctx.enter_context(tc.tile_pool(name="v", bufs=2))
        p_pool = ctx.enter_context(tc.tile_pool(name="p", bufs=34))
        rb_pool = ctx.enter_context(tc.tile_pool(name="rb", bufs=2))
        small = ctx.enter_context(tc.tile_pool(name="small", bufs=3))
        outp = ctx.enter_context(tc.tile_pool(name="outp", bufs=3))
        # one shared PSUM pool: 4 slots x 2 banks = all 8 banks
        psum = ctx.enter_context(tc.tile_pool(name="psum", bufs=4, space="PSUM"))

        lp = nc.allow_low_precision(reason="bf16 attention intermediates")
        lp.__enter__()

        # ---- constant tiles (DMAs emitted in the prologue, ordered so the
        # first score matmul's inputs land first) ----
        wq_sb = consts.tile([128, 2 * D_QK], bf16, tag="wq")
        wk_sb = consts.tile([128, 2 * D_QK], bf16, tag="wk")
        wv_sb = consts.tile([128, 2 * EV1], bf16, tag="wv")
        bq_sb = consts.tile([D_QK, 1], f32, tag="bq")
        bk_sb = consts.tile([D_QK, 1], f32, tag="bk")
        bv_sb = consts.tile([128, EV1], f32, tag="bv")
        ones_sb = consts.tile([1, D_V], bf16, tag="ones")

        def emit_qk_weight_dma():
            for kk in range(2):
                nc.sync.dma_start(
                    wq_sb[:, kk * D_QK:(kk + 1) * D_QK],
                    wq[kk * 128:(kk + 1) * 128, :])
                nc.sync.dma_start(
                    wk_sb[:, kk * D_QK:(kk + 1) * D_QK],
                    wk[kk * 128:(kk + 1) * 128, :])
            nc.sync.dma_start(bq_sb[:], bq[:, :])
            nc.sync.dma_start(bk_sb[:], bk[:, :])

        def emit_v_weight_dma():
            for kk in range(2):
                nc.sync.dma_start(
                    wv_sb[:, kk * EV1:(kk + 1) * EV1],
                    wv[kk * 128:(kk + 1) * 128, :])
            nc.sync.dma_start(bv_sb[:], bv[:, :])
            nc.sync.dma_start(ones_sb[:], ones[:, :])

        # per-batch SBUF tiles, allocated lazily
        xt_sb = [None] * BPC
        qtd = [None] * BPC
        ktd = [None] * BPC
        v_sb = [None] * BPC

        def alloc_xt(b):
            xt_sb[b] = xt_pool.tile([128, 2 * N], bf16, tag="xt", name=f"xtsb{b}")

        def emit_xt_dma(b, hh):
            for kk in range(2):
                nc.sync.dma_start(
                    xt_sb[b][:, kk * N + hh * NH: kk * N + (hh + 1) * NH],
                    xt[b, kk * 128:(kk + 1) * 128, hh * NH:(hh + 1) * NH])

        def qkv_units(b):
            """Projection work as small PE units (closures)."""
            qtd[b] = qk_pool.tile([128, N], bf16, tag="qt", name=f"qtd{b}")
            ktd[b] = qk_pool.tile([128, N], bf16, tag="kt", name=f"ktd{b}")
            v_sb[b] = v_pool.tile([128, 16 * EV1], bf16, tag="v", name=f"vsb{b}")
            units = []

            def qk_unit(w_sb, b_sb, dst, jh):
                def emit():
                    ps = psum.tile([D_QK, NH], f32, tag="big")
                    for kk in range(2):
                        for j in range(2):
                            nc.tensor.matmul(
                                ps[:, j * 512:(j + 1) * 512],
                                w_sb[:, kk * D_QK:(kk + 1) * D_QK],
                                xt_sb[b][:, kk * N + jh * NH + j * 512:
                                          kk * N + jh * NH + (j + 1) * 512],
                                start=(kk == 0), stop=(kk == 1))
                    for j in range(2):
                        jj = jh * NH + j * 512
                        # write both partition halves (row-packing duplicate)
                        # on the lightly-loaded DVE
                        nc.vector.tensor_scalar_add(
                            dst[0:D_QK, jj:jj + 512],
                            ps[:, j * 512:(j + 1) * 512], b_sb[:])
                        nc.vector.tensor_scalar_add(
                            dst[D_QK:128, jj:jj + 512],
                            ps[:, j * 512:(j + 1) * 512], b_sb[:])
                return emit

            def v_unit(m_lo):
                def emit():
                    for m in range(m_lo, m_lo + 4):
                        ps = psum.tile([128, EV1], f32, tag="big")
                        for kk in range(2):
                            nc.tensor.matmul(
                                ps[:],
                                xt_sb[b][:, kk * N + m * 128: kk * N + (m + 1) * 128],
                                wv_sb[:, kk * EV1:(kk + 1) * EV1],
                                start=(kk == 0), stop=(kk == 1))
                        nc.vector.tensor_add(
                            v_sb[b][:, m * EV1:(m + 1) * EV1], ps[:], bv_sb[:])
                return emit

            for jh in range(2):
                units.append(qk_unit(wq_sb, bq_sb, qtd[b], jh))
                units.append(qk_unit(wk_sb, bk_sb, ktd[b], jh))
            for m_lo in range(0, 16, 4):
                units.append(v_unit(m_lo))
            return units

        def emit_ot_mm(b, ot, mp, pjs):
            """One attention@V accumulation unit (4 matmuls, weight-shared)."""
            m0, m1 = 2 * mp, 2 * mp + 1
            p0, p1 = pjs
            nc.tensor.matmul(
                ot[:, 0:512], v_sb[b][:, m0 * EV1:(m0 + 1) * EV1],
                p0[:, 0:512],
                start=(mp == 0), stop=False, skip_group_check=True)
            nc.tensor.matmul(
                ot[:, 512:1024], v_sb[b][:, m0 * EV1:(m0 + 1) * EV1],
                p1[:, 0:512],
                start=(mp == 0), stop=False, skip_group_check=True)
            nc.tensor.matmul(
                ot[:, 0:512], v_sb[b][:, m1 * EV1:(m1 + 1) * EV1],
                p0[:, 512:1024],
                start=False, stop=(mp == 7), skip_group_check=True)
            nc.tensor.matmul(
                ot[:, 512:1024], v_sb[b][:, m1 * EV1:(m1 + 1) * EV1],
                p1[:, 512:1024],
                start=False, stop=(mp == 7), skip_group_check=True)

        def emit_epilogue(b, h, ot):
            den_sb = small.tile([1, NH], f32, tag="den")
            nc.vector.tensor_copy(den_sb[:], ot[D_V:EV1, :])
            rcp = small.tile([1, NH], f32, tag="rcp")
            nc.vector.reciprocal_approx_fast(rcp[:], den_sb[:])
            rcp16 = small.tile([1, NH], bf16, tag="rcp16")
            nc.vector.tensor_copy(rcp16[:], rcp[:])
            rb_sb = rb_pool.tile([D_V, NH], bf16, tag="rb")
            # broadcast 1/den across partitions on the idle GpSimd engine
            nc.gpsimd.partition_broadcast(rb_sb[:], rcp16[:])
            o_sb = outp.tile([D_V, NH], f32, tag="o")
            nc.vector.tensor_mul(o_sb[:], ot[0:D_V, :], rb_sb[:])
            nc.sync.dma_start(out[b, :, h * NH:(h + 1) * NH], o_sb[:])

        filler = deque()  # PE work to interleave into score/exp emission
        must_emit = deque()  # upcoming-batch QKV work (drained with priority)

        def pop_filler(k):
            for _ in range(k):
                if must_emit:
                    must_emit.popleft()()
                elif filler:
                    filler.popleft()()
                else:
                    break

        # ---- prologue: inputs + minimal projections before first scores ----
        # DMA order = dependency order of the first score matmul
        alloc_xt(0)
        emit_xt_dma(0, 0)  # x^T columns 0..1023, both d-halves
        emit_qk_weight_dma()
        # warm the exp table while the first DMAs fly
        warm = small.tile([1, 1], f32, tag="warm")
        nc.scalar.activation(warm[:], bq_sb[0:1, 0:1], Exp)
        u0 = qkv_units(0)
        u0[0]()  # Q columns 0..1023 (query half 0)
        u0[1]()  # K columns 0..1023 (key chunks 0..7)
        emit_xt_dma(0, 1)
        emit_v_weight_dma()
        if BPC > 1:
            alloc_xt(1)
            emit_xt_dma(1, 0)
            emit_xt_dma(1, 1)
        must_emit.extend(u0[2:])  # K hi, Q hi, V units — popped during half 0
        if BPC > 1:
            must_emit.extend(qkv_units(1))

        # ---- attention: exp-paced pipeline; attention@V lags by 2 pairs ----
        OT_LAG = 2
        for b in range(BPC):
            # this batch's projections must precede its scores on the PE
            # (batch 0's minimal prologue already ran; the rest is popped
            # as filler during half 0)
            while b > 0 and must_emit:
                must_emit.popleft()()
            for h in range(N // NH):
                ot = psum.tile([EV1, NH], f32, tag="big", name=f"ot{b}_{h}")
                p_tiles = []
                for mp in range(8):  # key-chunk pair (2*mp, 2*mp+1)
                    m0, m1 = 2 * mp, 2 * mp + 1
                    pjs = []
                    for js in range(2):  # 512-wide query slice in half
                        q0 = h * NH + js * 512
                        st = psum.tile([128, NH], f32, tag="big", name="st")
                        # row-packed pair: array rows 0-63 and 64-127
                        nc.tensor.matmul(
                            st[:, 0:512],
                            ktd[b][0:D_QK, m0 * 128:(m0 + 1) * 128],
                            qtd[b][0:D_QK, q0:q0 + 512],
                            start=True, stop=True)
                        nc.tensor.matmul(
                            st[:, 512:1024],
                            ktd[b][D_QK:128, m1 * 128:(m1 + 1) * 128],
                            qtd[b][D_QK:128, q0:q0 + 512],
                            start=True, stop=True)
                        p = p_pool.tile([128, NH], bf16, tag="p", name="p")
                        nc.scalar.activation(p[:], st[:], Exp)
                        pjs.append(p)
                    p_tiles.append(pjs)
                    if mp >= OT_LAG:
                        emit_ot_mm(b, ot, mp - OT_LAG, p_tiles[mp - OT_LAG])
                        pop_filler(1)
                    else:
                        pop_filler(2)
                # defer the last OT_LAG units + epilogue into the next half
                for mp_tail in range(8 - OT_LAG, 8):
                    filler.append(
                        (lambda bb=b, oo=ot, mm=mp_tail, pp=p_tiles[mp_tail]:
                         emit_ot_mm(bb, oo, mm, pp)))
                filler.append(lambda bb=b, hh=h, oo=ot: emit_epilogue(bb, hh, oo))

        while filler:
            filler.popleft()()

        lp.__exit__(None, None, None)

    nc.compile()
    return nc


def _get_nc():
    if "nc" not in _CACHE:
        _CACHE["nc"] = _build_nc()
    return _CACHE["nc"]


def kernel(x, wq, bq, wk, bk, wv, bv):
    global LAST_EXEC_TIME_NS
    import ml_dtypes
    from concourse.bass_utils import run_bass_kernel_spmd

    bf16 = ml_dtypes.bfloat16
    x = np.asarray(x, dtype=np.float32)
    wq = np.asarray(wq, dtype=np.float32)
    wk = np.asarray(wk, dtype=np.float32)
    wv = np.asarray(wv, dtype=np.float32)
    bq = np.asarray(bq, dtype=np.float32)
    bk = np.asarray(bk, dtype=np.float32)
    bv = np.asarray(bv, dtype=np.float32)

    # host-side layout/precision prep
    xt = np.ascontiguousarray(x.transpose(0, 2, 1)).astype(bf16)  # [B, D, N]
    wq8 = (wq / 8.0).astype(bf16)  # fold 1/sqrt(d_qk) into q projection
    bq8 = (bq / 8.0).astype(np.float32).reshape(D_QK, 1)
    wkb = wk.astype(bf16)
    bkb = bk.astype(np.float32).reshape(D_QK, 1)
    wv_aug = np.zeros((D_IN, EV1), np.float32)
    wv_aug[:, :D_V] = wv
    wv_augb = wv_aug.astype(bf16)
    bv_aug = np.zeros((128, EV1), np.float32)
    bv_aug[:, :D_V] = bv  # broadcast bias to all partitions
    bv_aug[:, D_V] = 1.0  # ones column -> softmax denominator
    ones = np.ones((1, D_V), bf16)

    in_maps = []
    for c in range(N_CORES):
        in_maps.append({
            "xt": np.ascontiguousarray(xt[BPC * c: BPC * (c + 1)]),
            "wq": wq8, "wk": wkb, "wv": wv_augb,
            "bq": bq8, "bk": bkb, "bv": bv_aug, "ones": ones,
        })

    nc = _get_nc()
    trace = bool(int(os.environ.get("ATTN_PROFILE", "0")))
    res = run_bass_kernel_spmd(
        nc, in_maps, core_ids=list(range(N_CORES)), trace=trace)
    LAST_EXEC_TIME_NS = res.exec_time_ns

    outs = np.stack([r["out"] for r in res.results])  # [8, BPC, 64, N]
    out = outs.reshape(B, D_V, N).transpose(0, 2, 1)  # [B, N, 64]
    return np.ascontiguousarray(out).astype(np.float32)
